# revision 3
# baseline (speedup 1.0000x reference)
"""Multi-head causal self-attention (B=2, T=2048, D=1024, H=16, Dh=64) on 8 TRN2 cores.

Sharding: data-parallel over batch (2 groups of 4 cores), tensor-parallel over
heads within a group (4 heads/core). Each core computes its 4 heads'
QKV projection + causal flash attention + its slice of the output projection;
the host sums the 4 partial outputs per batch.

Device layout notes (per core):
  - Host ships x[b] pre-transposed as xT [D, T] so the contraction dim (d)
    lands on SBUF partitions for all projection matmuls.
  - Attention runs in the "transposed" orientation: S^T[tk, tq] blocks, with
    softmax sums obtained by augmenting V with a ones column so the PV matmul
    emits [y^T; sums]. This keeps every matmul's moving dim at 512 and avoids
    any on-device transposes.
  - All matmuls use float32r (tf32-class, 1 cycle/row) — ~1.6e-4 rel error
    per matmul vs 4 cycles/row for exact fp32.
"""
import sys

import numpy as np

for _p in ("/opt/trn_rl_repo", "/root/.axon_site/_ro/trn_rl_repo"):
    if _p not in sys.path:
        try:
            import concourse  # noqa: F401
            break
        except ImportError:
            sys.path.append(_p)

import concourse.bass as bass  # noqa: E402
import concourse.tile as tile  # noqa: E402
from concourse import bacc, mybir  # noqa: E402
from concourse.bass_utils import run_bass_kernel_spmd  # noqa: E402

P = 128
T = 2048
D = 1024
NH = 4          # heads per core
DH = 64
F = NH * DH     # per-core head features (256)
DC = D // P     # 8 contraction chunks
TJ = T // 512   # 4 tq slices
TC = T // P     # 16 tk chunks
N_CORES = 8
FR = mybir.dt.float32r
F32 = mybir.dt.float32
AF = mybir.ActivationFunctionType


def build():
    nc = bacc.Bacc("TRN2", target_bir_lowering=False, debug=False, num_devices=N_CORES)
    xT = nc.dram_tensor("xT", [D, T], FR, kind="ExternalInput").ap()
    wqT = nc.dram_tensor("wqT", [D, F], FR, kind="ExternalInput").ap()
    wkT = nc.dram_tensor("wkT", [D, F], FR, kind="ExternalInput").ap()
    wvT = nc.dram_tensor("wvT", [D, F], FR, kind="ExternalInput").ap()
    woT = nc.dram_tensor("woT", [F, D], FR, kind="ExternalInput").ap()
    mask = nc.dram_tensor("mask", [P, 4 * 512], FR, kind="ExternalInput").ap()
    out = nc.dram_tensor("out", [T, D], F32, kind="ExternalOutput").ap()

    with tile.TileContext(nc) as tc:
        with (
            tc.tile_pool(name="weights", bufs=1) as wpool,
            tc.tile_pool(name="persist", bufs=1) as persist,
            tc.tile_pool(name="x", bufs=2) as xpool,
            tc.tile_pool(name="sexp", bufs=4) as sepool,
            tc.tile_pool(name="small", bufs=4) as small,
            tc.tile_pool(name="outsb", bufs=4) as opool,
            tc.tile_pool(name="ps_a", bufs=3, space="PSUM") as ps_a,
            tc.tile_pool(name="ps_y", bufs=2, space="PSUM") as ps_y,
            tc.tile_pool(name="ps_b", bufs=2, space="PSUM") as ps_b,
        ):
            wq_sb = wpool.tile([P, DC, F], FR)
            wk_sb = wpool.tile([P, DC, F], FR)
            wv_sb = wpool.tile([P, DC, F], FR)
            nc.sync.dma_start(wq_sb[:], wqT.rearrange("(o p) f -> p o f", p=P))
            nc.sync.dma_start(wk_sb[:], wkT.rearrange("(o p) f -> p o f", p=P))
            nc.sync.dma_start(wv_sb[:], wvT.rearrange("(o p) f -> p o f", p=P))
            wo_sb = wpool.tile([P, 2, D], FR)
            nc.sync.dma_start(wo_sb[:], woT.rearrange("(g p) e -> p g e", p=P))
            mask_sb = wpool.tile([P, 4, 512], FR)
            nc.sync.dma_start(mask_sb[:], mask.rearrange("p (r q) -> p r q", r=4))

            qT_sb = persist.tile([P, 2, T], FR)
            kT_sb = persist.tile([P, 2, T], FR)
            v_sb = persist.tile([P, NH, TC, DH + 1], FR)
            yT_sb = persist.tile([P, 2, T], FR)
            # All-ones constants carved out of the mask (memset can't write
            # float32r): mask[0, r=0, :] is all ones, as is mask[:, r=3, 511].
            ones_row = mask_sb[0:1, 0, 0:DH]
            nc.vector.tensor_copy(
                v_sb[:, :, :, DH : DH + 1],
                mask_sb[:, 3:4, 511:512, None].to_broadcast([P, NH, TC, 1]),
            )

            # ---- Phase 1: QKV projections (contract d on partitions) ----
            xT_r = xT.rearrange("(o p) t -> p o t", p=P)
            for j in range(TJ):
                x_sb = xpool.tile([P, DC, 512], FR, tag="x")
                nc.sync.dma_start(x_sb[:], xT_r[:, :, 512 * j : 512 * (j + 1)])
                for w_sb, dst in ((wq_sb, qT_sb), (wk_sb, kT_sb)):
                    for c in range(2):
                        pt = ps_a.tile([P, 512], F32, tag="a")
                        for o in range(DC):
                            nc.tensor.matmul(
                                pt[:],
                                w_sb[:, o, 128 * c : 128 * (c + 1)],
                                x_sb[:, o, :],
                                start=(o == 0),
                                stop=(o == DC - 1),
                            )
                        nc.scalar.copy(dst[:, c, 512 * j : 512 * (j + 1)], pt[:])
                for i in range(4):
                    pt = ps_a.tile([P, 512], F32, tag="a")
                    for o in range(DC):
                        nc.tensor.matmul(
                            pt[:, :F],
                            x_sb[:, o, 128 * i : 128 * (i + 1)],
                            wv_sb[:, o, :],
                            start=(o == 0),
                            stop=(o == DC - 1),
                        )
                    nc.scalar.copy(
                        v_sb[:, :, 4 * j + i, 0:DH],
                        pt[:, :F].rearrange("p (h d) -> p h d", h=NH),
                    )

            # ---- Phase 2: causal attention, transposed orientation ----
            scale = 1.0 / np.sqrt(DH)
            for h in range(NH):
                hc, hp = h // 2, (h % 2) * DH
                for j in range(TJ):
                    yps = ps_y.tile([DH + 1, 512], F32, tag="y")
                    nii = 4 * j + 4
                    for ii in range(nii):
                        sps = ps_a.tile([P, 512], F32, tag="a")
                        nc.tensor.matmul(
                            sps[:],
                            kT_sb[hp : hp + DH, hc, 128 * ii : 128 * (ii + 1)],
                            qT_sb[hp : hp + DH, hc, 512 * j : 512 * (j + 1)],
                            start=True,
                            stop=True,
                        )
                        se = sepool.tile([P, 512], FR, tag="se")
                        nc.scalar.activation(se[:], sps[:], AF.Exp, scale=scale)
                        r = ii - 4 * j
                        if r >= 0:
                            nc.vector.tensor_mul(se[:], se[:], mask_sb[:, r, :])
                        nc.tensor.matmul(
                            yps[:],
                            v_sb[:, h, ii, :],
                            se[:],
                            start=(ii == 0),
                            stop=(ii == nii - 1),
                        )
                    rec = small.tile([1, 512], FR, tag="rec")
                    with nc.allow_low_precision(reason="softmax recip feeds f32r matmul"):
                        nc.vector.reciprocal(rec[:], yps[DH : DH + 1, :])
                    bps = ps_b.tile([DH, 512], F32, tag="b")
                    nc.tensor.matmul(bps[:], ones_row, rec[:], start=True, stop=True)
                    bsb = small.tile([DH, 512], F32, tag="bsb")
                    nc.scalar.copy(bsb[:], bps[:])
                    nc.vector.tensor_mul(
                        yT_sb[hp : hp + DH, hc, 512 * j : 512 * (j + 1)],
                        yps[0:DH, :],
                        bsb[:],
                    )

            # ---- Phase 3: output projection (partial over this core's 256 feats) ----
            for tb in range(TC):
                for eb in range(2):
                    pt = ps_a.tile([P, 512], F32, tag="a")
                    for g in range(2):
                        nc.tensor.matmul(
                            pt[:],
                            yT_sb[:, g, 128 * tb : 128 * (tb + 1)],
                            wo_sb[:, g, 512 * eb : 512 * (eb + 1)],
                            start=(g == 0),
                            stop=(g == 1),
                        )
                    osb = opool.tile([P, 512], F32, tag="osb")
                    nc.vector.tensor_copy(osb[:], pt[:])
                    nc.sync.dma_start(
                        out[128 * tb : 128 * (tb + 1), 512 * eb : 512 * (eb + 1)],
                        osb[:],
                    )
    nc.compile()
    return nc


def make_mask() -> np.ndarray:
    q = np.arange(512)[None, None, :]
    p = np.arange(P)[:, None, None]
    r = np.arange(4)[None, :, None]
    m = (q >= 128 * r + p).astype(np.float32)
    return np.ascontiguousarray(m.reshape(P, 4 * 512))


def shard_inputs(x, Wqkv, Wout):
    mask = make_mask()
    in_maps = []
    for c in range(N_CORES):
        b, g = c // 4, c % 4
        sl = slice(F * g, F * (g + 1))
        in_maps.append(
            {
                "xT": np.ascontiguousarray(x[b].T),
                "wqT": np.ascontiguousarray(Wqkv[sl, :].T),
                "wkT": np.ascontiguousarray(Wqkv[D:][sl, :].T),
                "wvT": np.ascontiguousarray(Wqkv[2 * D:][sl, :].T),
                "woT": np.ascontiguousarray(Wout[:, sl].T),
                "mask": mask,
            }
        )
    return in_maps


_NC_CACHE = None


def kernel(x, Wqkv, Wout):
    global _NC_CACHE
    x = np.asarray(x, dtype=np.float32)
    Wqkv = np.asarray(Wqkv, dtype=np.float32)
    Wout = np.asarray(Wout, dtype=np.float32)
    if _NC_CACHE is None:
        _NC_CACHE = build()
    nc = _NC_CACHE
    in_maps = shard_inputs(x, Wqkv, Wout)
    res = run_bass_kernel_spmd(nc, in_maps, core_ids=list(range(N_CORES)))
    outs = [res.results[c]["out"] for c in range(N_CORES)]
    return np.stack(
        [outs[0] + outs[1] + outs[2] + outs[3], outs[4] + outs[5] + outs[6] + outs[7]]
    )


# revision 11
# speedup vs baseline: 1.4118x; 1.4118x over previous
"""Multi-head causal self-attention (B=2, T=2048, D=1024, H=16, Dh=64) on 8 TRN2 cores.

Sharding: data-parallel over batch (2 groups of 4 cores), tensor-parallel over
heads within a group (4 heads/core). Each core computes its 4 heads'
QKV projection + causal flash attention + its slice of the output projection;
the host sums the 4 partial outputs per batch.

Device design (per core):
  - Host ships x[b] pre-transposed as xT [D, T] so the contraction dim (d)
    lands on SBUF partitions for all projection matmuls.
  - Attention runs in the "transposed" orientation: S^T[tk, tq] blocks with
    tq as the moving dim (512), softmax sums via a ones-column appended to V
    so the PV matmul emits [y^T; sums]. No on-device transposes anywhere.
  - All matmuls are float32r (tf32-class, 1 cycle/row at N>=256).
  - Pipeline is tq-slice-major: project slice j, attend slice j (all heads),
    project slice j's output — so PE projection work overlaps ACT exp work.
  - Diagonal S^T blocks are column-sliced to the causal region; only the
    mixed 128x128 sub-block gets a mask multiply.
  - Head pairs (base partitions 0/64) issue adjacent S^T matmuls so the PE
    runs them concurrently in disjoint row groups.
"""
import sys

import numpy as np

for _p in ("/opt/trn_rl_repo", "/root/.axon_site/_ro/trn_rl_repo"):
    if _p not in sys.path:
        try:
            import concourse  # noqa: F401
            break
        except ImportError:
            sys.path.append(_p)

import concourse.bass as bass  # noqa: E402
import concourse.tile as tile  # noqa: E402
from concourse import bacc, mybir  # noqa: E402
from concourse.bass_utils import run_bass_kernel_spmd  # noqa: E402

P = 128
T = 2048
D = 1024
NH = 4          # heads per core
DH = 64
F = NH * DH     # per-core head features (256)
DC = D // P     # 8 contraction chunks
TJ = T // 512   # 4 tq slices
TC = T // P     # 16 tk chunks
N_CORES = 8
FR = mybir.dt.float32r
F32 = mybir.dt.float32
AF = mybir.ActivationFunctionType


def build():
    nc = bacc.Bacc("TRN2", target_bir_lowering=False, debug=False, num_devices=N_CORES)
    xT = nc.dram_tensor("xT", [D, T], FR, kind="ExternalInput").ap()
    wqT = nc.dram_tensor("wqT", [D, F], FR, kind="ExternalInput").ap()
    wkT = nc.dram_tensor("wkT", [D, F], FR, kind="ExternalInput").ap()
    wvT = nc.dram_tensor("wvT", [D, F], FR, kind="ExternalInput").ap()
    woT = nc.dram_tensor("woT", [F, D], FR, kind="ExternalInput").ap()
    mask = nc.dram_tensor("mask", [P, 4 * 512], FR, kind="ExternalInput").ap()
    out = nc.dram_tensor("out", [T, D], F32, kind="ExternalOutput").ap()

    scale = 1.0 / np.sqrt(DH)

    with tile.TileContext(nc) as tc:
        with (
            tc.tile_pool(name="weights", bufs=1) as wpool,
            tc.tile_pool(name="persist", bufs=1) as persist,
            tc.tile_pool(name="x", bufs=2) as xpool,
            tc.tile_pool(name="sexp", bufs=6) as sepool,
            tc.tile_pool(name="small", bufs=4) as small,
            tc.tile_pool(name="outsb", bufs=4) as opool,
            tc.tile_pool(name="ps_s", bufs=3, space="PSUM") as ps_s,
            tc.tile_pool(name="ps_y", bufs=2, space="PSUM") as ps_y,
            tc.tile_pool(name="ps_ao", bufs=2, space="PSUM") as ps_ao,
            tc.tile_pool(name="ps_b", bufs=1, space="PSUM") as ps_b,
        ):
            wq_sb = wpool.tile([P, DC, F], FR)
            wk_sb = wpool.tile([P, DC, F], FR)
            wv_sb = wpool.tile([P, DC, F], FR)
            wo_sb = wpool.tile([P, 2, D], FR)
            mask_sb = wpool.tile([P, 4, 512], FR)
            wq_r = wqT.rearrange("(o p) f -> p o f", p=P)
            wk_r = wkT.rearrange("(o p) f -> p o f", p=P)
            wv_r = wvT.rearrange("(o p) f -> p o f", p=P)
            wo_r = woT.rearrange("(g p) e -> p g e", p=P)
            xT_r = xT.rearrange("(o p) t -> p o t", p=P)

            qT_sb = persist.tile([P, 2, T], FR)
            kT_sb = persist.tile([P, 2, T], FR)
            v_sb = persist.tile([P, NH, TC, DH + 1], FR)
            yT_sb = persist.tile([P, 2, T], FR)

            x_tiles = {}

            def load_x(j):
                x_sb = xpool.tile([P, DC, 512], FR, tag="x", name=f"x_{j}")
                for o in range(DC):
                    nc.sync.dma_start(x_sb[:, o], xT_r[:, o, 512 * j : 512 * (j + 1)])
                x_tiles[j] = x_sb

            # First projection's operands go first on the DMA queues, chunk
            # interleaved, so the first q matmul can start after one x+wq
            # chunk pair instead of after the whole 3MB preload.
            x0_sb = xpool.tile([P, DC, 512], FR, tag="x", name="x_0")
            x_tiles[0] = x0_sb
            for o in range(DC):
                nc.sync.dma_start(x0_sb[:, o], xT_r[:, o, 0:512])
                nc.sync.dma_start(wq_sb[:, o], wq_r[:, o])
            for o in range(DC):
                nc.sync.dma_start(wk_sb[:, o], wk_r[:, o])
            for o in range(DC):
                nc.sync.dma_start(wv_sb[:, o], wv_r[:, o])
            nc.sync.dma_start(mask_sb[:], mask.rearrange("p (r q) -> p r q", r=4))
            load_x(1)
            for g in range(2):
                nc.sync.dma_start(wo_sb[:, g], wo_r[:, g])

            # All-ones constants carved out of the mask (memset can't write
            # float32r): mask[0, r=0, :] is all ones, as is mask[:, r=3, 511].
            ones_row = mask_sb[0:1, 0, 0:DH]
            nc.vector.tensor_copy(
                v_sb[:, :, :, DH : DH + 1],
                mask_sb[:, 3:4, 511:512, None].to_broadcast([P, NH, TC, 1]),
            )

            def proj(j):
                jsl = slice(512 * j, 512 * (j + 1))
                x_sb = x_tiles[j]
                for w_sb, dst in ((wq_sb, qT_sb), (wk_sb, kT_sb)):
                    for c in range(2):
                        pt = ps_ao.tile([P, 512], F32, tag="ao", name=f"qk_{j}_{c}")
                        for o in range(DC):
                            nc.tensor.matmul(
                                pt[:],
                                w_sb[:, o, 128 * c : 128 * (c + 1)],
                                x_sb[:, o, :],
                                start=(o == 0),
                                stop=(o == DC - 1),
                            )
                        nc.scalar.copy(dst[:, c, jsl], pt[:])
                for i in range(4):
                    pt = ps_ao.tile([P, 512], F32, tag="ao", name=f"v_{j}_{i}")
                    for o in range(DC):
                        nc.tensor.matmul(
                            pt[:, :F],
                            x_sb[:, o, 128 * i : 128 * (i + 1)],
                            wv_sb[:, o, :],
                            start=(o == 0),
                            stop=(o == DC - 1),
                        )
                    nc.vector.tensor_copy(
                        v_sb[:, :, 4 * j + i, 0:DH],
                        pt[:, :F].rearrange("p (h d) -> p h d", h=NH),
                    )

            def outproj(j):
                for tb in range(4 * j, 4 * (j + 1)):
                    for eb in range(2):
                        pt = ps_ao.tile([P, 512], F32, tag="ao", name=f"o_{tb}_{eb}")
                        for g in range(2):
                            nc.tensor.matmul(
                                pt[:],
                                yT_sb[:, g, 128 * tb : 128 * (tb + 1)],
                                wo_sb[:, g, 512 * eb : 512 * (eb + 1)],
                                start=(g == 0),
                                stop=(g == 1),
                            )
                        osb = opool.tile([P, 512], F32, tag="osb", name=f"osb_{tb}_{eb}")
                        nc.vector.tensor_copy(osb[:], pt[:])
                        nc.sync.dma_start(
                            out[128 * tb : 128 * (tb + 1), 512 * eb : 512 * (eb + 1)],
                            osb[:],
                        )

            proj(0)
            for j in range(TJ):
                jsl = slice(512 * j, 512 * (j + 1))

                # ---- attention for tq-slice j, head pairs interleaved ----
                nii = 4 * j + 4
                for c in range(2):  # head pair (2c, 2c+1)
                    ypair = [
                        ps_y.tile([DH + 1, 512], F32, tag="y", name=f"y_{j}_{c}_{t}")
                        for t in range(2)
                    ]
                    for ii in range(nii):
                        r = ii - 4 * j
                        # Partial-column S^T matmul + exp skip the fully-masked
                        # left columns. j==0 stays full width so the first use
                        # of every se/sps pool slot writes finite data (stale
                        # slot contents are later multiplied by the 0-mask, so
                        # they must never be NaN). PV accumulation must be
                        # full width: a start=False matmul over a column
                        # subrange of a PSUM bank kills the exec unit.
                        col0 = 128 * r if (r > 0 and j > 0) else 0
                        sepair = []
                        for t in range(2):
                            hp = 64 * t
                            sps = ps_s.tile([P, 512], F32, tag="s")
                            nc.tensor.matmul(
                                sps[:, col0:],
                                kT_sb[hp : hp + DH, c, 128 * ii : 128 * (ii + 1)],
                                qT_sb[hp : hp + DH, c, 512 * j + col0 : 512 * (j + 1)],
                                start=True,
                                stop=True,
                            )
                            se = sepool.tile([P, 512], FR, tag="se")
                            nc.scalar.activation(
                                se[:, col0:], sps[:, col0:], AF.Exp, scale=scale
                            )
                            if col0 > 0:
                                # Fully-masked left columns: write explicit
                                # zeros (mask x 0) so every column the PV
                                # matmul reads was produced this iteration.
                                nc.vector.tensor_scalar_mul(
                                    se[:, 0:col0], mask_sb[:, 0, 0:col0], 0.0
                                )
                                nc.vector.tensor_mul(
                                    se[:, col0 : col0 + 128],
                                    se[:, col0 : col0 + 128],
                                    mask_sb[:, r, col0 : col0 + 128],
                                )
                            elif r >= 0:
                                # Full-width exp (j==0 or r==0): mask away
                                # everything left of the diagonal block.
                                nc.vector.tensor_mul(
                                    se[:, 0 : 128 * (r + 1)],
                                    se[:, 0 : 128 * (r + 1)],
                                    mask_sb[:, r, 0 : 128 * (r + 1)],
                                )
                            sepair.append(se)
                        for t in range(2):
                            h = 2 * c + t
                            nc.tensor.matmul(
                                ypair[t][:],
                                v_sb[:, h, ii, :],
                                sepair[t][:],
                                start=(ii == 0),
                                stop=(ii == nii - 1),
                            )
                    for t in range(2):
                        hp = 64 * t
                        yps = ypair[t]
                        rec = small.tile([1, 512], FR, tag="rec")
                        with nc.allow_low_precision(reason="softmax recip feeds f32r matmul"):
                            nc.vector.reciprocal(rec[:], yps[DH : DH + 1, :])
                        bps = ps_b.tile([DH, 512], F32, tag="b")
                        nc.tensor.matmul(bps[:], ones_row, rec[:], start=True, stop=True)
                        bsb = small.tile([DH, 512], F32, tag="bsb")
                        nc.vector.tensor_copy(bsb[:], bps[:])
                        nc.vector.tensor_mul(yT_sb[hp : hp + DH, c, jsl], yps[0:DH, :], bsb[:])

                # Next slice's projection goes ahead of the output projections:
                # it has no dependency on the norm chain above, so the PE has
                # work while the DVE finishes normalizing. Output projection of
                # slice j-1 (deps long satisfied) fills the same window; slice
                # j's own out-proj lands one iteration later.
                if j + 1 < TJ:
                    if j + 2 < TJ:
                        load_x(j + 2)
                    proj(j + 1)
                if j > 0:
                    outproj(j - 1)
                if j == TJ - 1:
                    outproj(j)
    nc.compile()
    return nc


def make_mask() -> np.ndarray:
    q = np.arange(512)[None, None, :]
    p = np.arange(P)[:, None, None]
    r = np.arange(4)[None, :, None]
    m = (q >= 128 * r + p).astype(np.float32)
    return np.ascontiguousarray(m.reshape(P, 4 * 512))


def shard_inputs(x, Wqkv, Wout):
    mask = make_mask()
    in_maps = []
    for c in range(N_CORES):
        b, g = c // 4, c % 4
        sl = slice(F * g, F * (g + 1))
        in_maps.append(
            {
                "xT": np.ascontiguousarray(x[b].T),
                "wqT": np.ascontiguousarray(Wqkv[sl, :].T),
                "wkT": np.ascontiguousarray(Wqkv[D:][sl, :].T),
                "wvT": np.ascontiguousarray(Wqkv[2 * D:][sl, :].T),
                "woT": np.ascontiguousarray(Wout[:, sl].T),
                "mask": mask,
            }
        )
    return in_maps


_NC_CACHE = None


def kernel(x, Wqkv, Wout):
    global _NC_CACHE
    x = np.asarray(x, dtype=np.float32)
    Wqkv = np.asarray(Wqkv, dtype=np.float32)
    Wout = np.asarray(Wout, dtype=np.float32)
    if _NC_CACHE is None:
        _NC_CACHE = build()
    nc = _NC_CACHE
    in_maps = shard_inputs(x, Wqkv, Wout)
    res = run_bass_kernel_spmd(nc, in_maps, core_ids=list(range(N_CORES)))
    outs = [res.results[c]["out"] for c in range(N_CORES)]
    return np.stack(
        [outs[0] + outs[1] + outs[2] + outs[3], outs[4] + outs[5] + outs[6] + outs[7]]
    )


# revision 19
# speedup vs baseline: 1.4267x; 1.0105x over previous
"""Multi-head causal self-attention (B=2, T=2048, D=1024, H=16, Dh=64) on 8 TRN2 cores.

Sharding: data-parallel over batch (2 groups of 4 cores), tensor-parallel over
heads within a group (4 heads/core). Each core computes its 4 heads'
QKV projection + causal flash attention + its slice of the output projection;
the host sums the 4 partial outputs per batch.

Device design (per core):
  - Host ships x[b] pre-transposed as xT [D, T] so the contraction dim (d)
    lands on SBUF partitions for all projection matmuls.
  - Attention runs in the "transposed" orientation: S^T[tk, tq] blocks with
    tq as the moving dim (512), softmax sums via a ones-column appended to V
    so the PV matmul emits [y^T; sums]. No on-device transposes anywhere.
  - All matmuls are float32r (tf32-class, 1 cycle/row at N>=256).
  - Pipeline is tq-slice-major: project slice j, attend slice j (all heads),
    project slice j's output — so PE projection work overlaps ACT exp work.
  - Diagonal S^T blocks are column-sliced to the causal region; only the
    mixed 128x128 sub-block gets a mask multiply.
  - Head pairs (base partitions 0/64) issue adjacent S^T matmuls so the PE
    runs them concurrently in disjoint row groups.
"""
import sys

import numpy as np

for _p in ("/opt/trn_rl_repo", "/root/.axon_site/_ro/trn_rl_repo"):
    if _p not in sys.path:
        try:
            import concourse  # noqa: F401
            break
        except ImportError:
            sys.path.append(_p)

import concourse.bass as bass  # noqa: E402
import concourse.tile as tile  # noqa: E402
from concourse import bacc, mybir  # noqa: E402
from concourse.bass_utils import run_bass_kernel_spmd  # noqa: E402

P = 128
T = 2048
D = 1024
NH = 4          # heads per core
DH = 64
F = NH * DH     # per-core head features (256)
DC = D // P     # 8 contraction chunks
TJ = T // 512   # 4 tq slices
TC = T // P     # 16 tk chunks
N_CORES = 8
FR = mybir.dt.float32r
F32 = mybir.dt.float32
AF = mybir.ActivationFunctionType


def build():
    nc = bacc.Bacc("TRN2", target_bir_lowering=False, debug=False, num_devices=N_CORES)
    xT = nc.dram_tensor("xT", [D, T], FR, kind="ExternalInput").ap()
    wqT = nc.dram_tensor("wqT", [D, F], FR, kind="ExternalInput").ap()
    wkT = nc.dram_tensor("wkT", [D, F], FR, kind="ExternalInput").ap()
    wvT = nc.dram_tensor("wvT", [D, F], FR, kind="ExternalInput").ap()
    woT = nc.dram_tensor("woT", [F, D], FR, kind="ExternalInput").ap()
    mask = nc.dram_tensor("mask", [P, 4 * 512], FR, kind="ExternalInput").ap()
    out = nc.dram_tensor("out", [T, D], F32, kind="ExternalOutput").ap()

    scale = 1.0 / np.sqrt(DH)

    with tile.TileContext(nc) as tc:
        with (
            tc.tile_pool(name="weights", bufs=1) as wpool,
            tc.tile_pool(name="persist", bufs=1) as persist,
            tc.tile_pool(name="x", bufs=2) as xpool,
            tc.tile_pool(name="sexp", bufs=6) as sepool,
            tc.tile_pool(name="small", bufs=4) as small,
            tc.tile_pool(name="outsb", bufs=4) as opool,
            tc.tile_pool(name="ps_s", bufs=3, space="PSUM") as ps_s,
            tc.tile_pool(name="ps_y", bufs=3, space="PSUM") as ps_y,
            tc.tile_pool(name="ps_ao", bufs=2, space="PSUM") as ps_ao,
        ):
            wq_sb = wpool.tile([P, DC, F], FR)
            wk_sb = wpool.tile([P, DC, F], FR)
            wv_sb = wpool.tile([P, DC, F], FR)
            wo_sb = wpool.tile([P, 2, D], FR)
            mask_sb = wpool.tile([P, 4, 512], FR)
            wq_r = wqT.rearrange("(o p) f -> p o f", p=P)
            wk_r = wkT.rearrange("(o p) f -> p o f", p=P)
            wv_r = wvT.rearrange("(o p) f -> p o f", p=P)
            wo_r = woT.rearrange("(g p) e -> p g e", p=P)
            xT_r = xT.rearrange("(o p) t -> p o t", p=P)

            qT_sb = persist.tile([P, 2, T], FR)
            kT_sb = persist.tile([P, 2, T], FR)
            v_sb = persist.tile([P, NH, TC, DH + 1], FR)
            yT_sb = persist.tile([P, 2, T], FR)

            x_tiles = {}

            def load_x(j):
                x_sb = xpool.tile([P, DC, 512], FR, tag="x", name=f"x_{j}")
                for o in range(DC):
                    nc.sync.dma_start(x_sb[:, o], xT_r[:, o, 512 * j : 512 * (j + 1)])
                x_tiles[j] = x_sb

            # First projection's operands go first on the DMA queues, chunk
            # interleaved, so the first q matmul can start after one x+wq
            # chunk pair instead of after the whole 3MB preload.
            x0_sb = xpool.tile([P, DC, 512], FR, tag="x", name="x_0")
            x_tiles[0] = x0_sb
            for o in range(DC):
                nc.sync.dma_start(x0_sb[:, o], xT_r[:, o, 0:512])
                nc.sync.dma_start(wq_sb[:, o], wq_r[:, o])
            for o in range(DC):
                nc.sync.dma_start(wk_sb[:, o], wk_r[:, o])
            for o in range(DC):
                nc.sync.dma_start(wv_sb[:, o], wv_r[:, o])
            nc.sync.dma_start(mask_sb[:], mask.rearrange("p (r q) -> p r q", r=4))
            load_x(1)
            for g in range(2):
                nc.sync.dma_start(wo_sb[:, g], wo_r[:, g])

            # V's ones column comes from the mask (memset can't write
            # float32r): mask[:, r=3, 511] is all ones.
            nc.vector.tensor_copy(
                v_sb[:, :, :, DH : DH + 1],
                mask_sb[:, 3:4, 511:512, None].to_broadcast([P, NH, TC, 1]),
            )

            def proj(j):
                jsl = slice(512 * j, 512 * (j + 1))
                x_sb = x_tiles[j]
                for wi, (w_sb, dst) in enumerate(((wq_sb, qT_sb), (wk_sb, kT_sb))):
                    # Both column chunks advance together so each arriving
                    # x/w DMA chunk feeds two back-to-back matmuls.
                    pts = [
                        ps_ao.tile([P, 512], F32, tag="ao", name=f"qk_{j}_{wi}_{c}")
                        for c in range(2)
                    ]
                    for o in range(DC):
                        for c in range(2):
                            nc.tensor.matmul(
                                pts[c][:],
                                w_sb[:, o, 128 * c : 128 * (c + 1)],
                                x_sb[:, o, :],
                                start=(o == 0),
                                stop=(o == DC - 1),
                            )
                    for c in range(2):
                        nc.scalar.copy(dst[:, c, jsl], pts[c][:])
                for i in range(4):
                    pt = ps_ao.tile([P, 512], F32, tag="ao", name=f"v_{j}_{i}")
                    for o in range(DC):
                        nc.tensor.matmul(
                            pt[:, :F],
                            x_sb[:, o, 128 * i : 128 * (i + 1)],
                            wv_sb[:, o, :],
                            start=(o == 0),
                            stop=(o == DC - 1),
                        )
                    nc.vector.tensor_copy(
                        v_sb[:, :, 4 * j + i, 0:DH],
                        pt[:, :F].rearrange("p (h d) -> p h d", h=NH),
                    )

            def outproj(j):
                for tb in range(4 * j, 4 * (j + 1)):
                    for eb in range(2):
                        pt = ps_ao.tile([P, 512], F32, tag="ao", name=f"o_{tb}_{eb}")
                        for g in range(2):
                            nc.tensor.matmul(
                                pt[:],
                                yT_sb[:, g, 128 * tb : 128 * (tb + 1)],
                                wo_sb[:, g, 512 * eb : 512 * (eb + 1)],
                                start=(g == 0),
                                stop=(g == 1),
                            )
                        osb = opool.tile([P, 512], F32, tag="osb", name=f"osb_{tb}_{eb}")
                        nc.vector.tensor_copy(osb[:], pt[:])
                        nc.sync.dma_start(
                            out[128 * tb : 128 * (tb + 1), 512 * eb : 512 * (eb + 1)],
                            osb[:],
                        )

            proj(0)
            for j in range(TJ):
                jsl = slice(512 * j, 512 * (j + 1))

                # ---- attention for tq-slice j, head pairs interleaved ----
                nii = 4 * j + 4
                for c in range(2):  # head pair (2c, 2c+1)
                    ypair = [
                        ps_y.tile([DH + 1, 512], F32, tag="y", name=f"y_{j}_{c}_{t}")
                        for t in range(2)
                    ]
                    for ii in range(nii):
                        r = ii - 4 * j
                        # Partial-column S^T matmul + exp skip the fully-masked
                        # left columns. j==0 stays full width so the first use
                        # of every se/sps pool slot writes finite data (stale
                        # slot contents are later multiplied by the 0-mask, so
                        # they must never be NaN). PV accumulation must be
                        # full width: a start=False matmul over a column
                        # subrange of a PSUM bank kills the exec unit.
                        col0 = 128 * r if (r > 0 and j > 0) else 0
                        sepair = []
                        for t in range(2):
                            hp = 64 * t
                            sps = ps_s.tile([P, 512], F32, tag="s")
                            nc.tensor.matmul(
                                sps[:, col0:],
                                kT_sb[hp : hp + DH, c, 128 * ii : 128 * (ii + 1)],
                                qT_sb[hp : hp + DH, c, 512 * j + col0 : 512 * (j + 1)],
                                start=True,
                                stop=True,
                            )
                            se = sepool.tile([P, 512], FR, tag="se")
                            nc.scalar.activation(
                                se[:, col0:], sps[:, col0:], AF.Exp, scale=scale
                            )
                            if col0 > 0:
                                # Fully-masked left columns: write explicit
                                # zeros (mask x 0) so every column the PV
                                # matmul reads was produced this iteration.
                                nc.vector.tensor_scalar_mul(
                                    se[:, 0:col0], mask_sb[:, 0, 0:col0], 0.0
                                )
                                nc.vector.tensor_mul(
                                    se[:, col0 : col0 + 128],
                                    se[:, col0 : col0 + 128],
                                    mask_sb[:, r, col0 : col0 + 128],
                                )
                            elif r >= 0:
                                # Full-width exp (j==0 or r==0): mask away
                                # everything left of the diagonal block.
                                nc.vector.tensor_mul(
                                    se[:, 0 : 128 * (r + 1)],
                                    se[:, 0 : 128 * (r + 1)],
                                    mask_sb[:, r, 0 : 128 * (r + 1)],
                                )
                            sepair.append(se)
                        for t in range(2):
                            h = 2 * c + t
                            nc.tensor.matmul(
                                ypair[t][:],
                                v_sb[:, h, ii, :],
                                sepair[t][:],
                                start=(ii == 0),
                                stop=(ii == nii - 1),
                            )
                    for t in range(2):
                        hp = 64 * t
                        yps = ypair[t]
                        rec = small.tile([1, 512], F32, tag="rec")
                        nc.vector.reciprocal(rec[:], yps[DH : DH + 1, :])
                        bsb = small.tile([DH, 512], F32, tag="bsb")
                        nc.gpsimd.partition_broadcast(bsb[:], rec[:])
                        nc.vector.tensor_mul(yT_sb[hp : hp + DH, c, jsl], yps[0:DH, :], bsb[:])

                # Next slice's projection goes ahead of the output projections:
                # it has no dependency on the norm chain above, so the PE has
                # work while the DVE finishes normalizing. Output projection of
                # slice j-1 (deps long satisfied) fills the same window; slice
                # j's own out-proj lands one iteration later.
                if j + 1 < TJ:
                    if j + 2 < TJ:
                        load_x(j + 2)
                    proj(j + 1)
                if j > 0:
                    outproj(j - 1)
                if j == TJ - 1:
                    outproj(j)
    nc.compile()
    return nc


def make_mask() -> np.ndarray:
    q = np.arange(512)[None, None, :]
    p = np.arange(P)[:, None, None]
    r = np.arange(4)[None, :, None]
    m = (q >= 128 * r + p).astype(np.float32)
    return np.ascontiguousarray(m.reshape(P, 4 * 512))


def shard_inputs(x, Wqkv, Wout):
    mask = make_mask()
    in_maps = []
    for c in range(N_CORES):
        b, g = c // 4, c % 4
        sl = slice(F * g, F * (g + 1))
        in_maps.append(
            {
                "xT": np.ascontiguousarray(x[b].T),
                "wqT": np.ascontiguousarray(Wqkv[sl, :].T),
                "wkT": np.ascontiguousarray(Wqkv[D:][sl, :].T),
                "wvT": np.ascontiguousarray(Wqkv[2 * D:][sl, :].T),
                "woT": np.ascontiguousarray(Wout[:, sl].T),
                "mask": mask,
            }
        )
    return in_maps


_NC_CACHE = None


def kernel(x, Wqkv, Wout):
    global _NC_CACHE
    x = np.asarray(x, dtype=np.float32)
    Wqkv = np.asarray(Wqkv, dtype=np.float32)
    Wout = np.asarray(Wout, dtype=np.float32)
    if _NC_CACHE is None:
        _NC_CACHE = build()
    nc = _NC_CACHE
    in_maps = shard_inputs(x, Wqkv, Wout)
    res = run_bass_kernel_spmd(nc, in_maps, core_ids=list(range(N_CORES)))
    outs = [res.results[c]["out"] for c in range(N_CORES)]
    return np.stack(
        [outs[0] + outs[1] + outs[2] + outs[3], outs[4] + outs[5] + outs[6] + outs[7]]
    )


# revision 25
# speedup vs baseline: 1.4279x; 1.0009x over previous
"""Multi-head causal self-attention (B=2, T=2048, D=1024, H=16, Dh=64) on 8 TRN2 cores.

Sharding: data-parallel over batch (2 groups of 4 cores), tensor-parallel over
heads within a group (4 heads/core). Each core computes its 4 heads'
QKV projection + causal flash attention + its slice of the output projection;
the host sums the 4 partial outputs per batch.

Device design (per core):
  - Host ships x[b] pre-transposed as xT [D, T] so the contraction dim (d)
    lands on SBUF partitions for all projection matmuls.
  - Attention runs in the "transposed" orientation: S^T[tk, tq] blocks with
    tq as the moving dim (512), softmax sums via a ones-column appended to V
    so the PV matmul emits [y^T; sums]. No on-device transposes anywhere.
  - All matmuls are float32r (tf32-class, 1 cycle/row at N>=256).
  - Pipeline is tq-slice-major: project slice j, attend slice j (all heads),
    project slice j's output — so PE projection work overlaps ACT exp work.
  - Diagonal S^T blocks are column-sliced to the causal region; only the
    mixed 128x128 sub-block gets a mask multiply.
  - Head pairs (base partitions 0/64) issue adjacent S^T matmuls so the PE
    runs them concurrently in disjoint row groups.
"""
import sys

import numpy as np

for _p in ("/opt/trn_rl_repo", "/root/.axon_site/_ro/trn_rl_repo"):
    if _p not in sys.path:
        try:
            import concourse  # noqa: F401
            break
        except ImportError:
            sys.path.append(_p)

import concourse.bass as bass  # noqa: E402
import concourse.tile as tile  # noqa: E402
from concourse import bacc, mybir  # noqa: E402
from concourse.bass_utils import run_bass_kernel_spmd  # noqa: E402

P = 128
T = 2048
D = 1024
NH = 4          # heads per core
DH = 64
F = NH * DH     # per-core head features (256)
DC = D // P     # 8 contraction chunks
TJ = T // 512   # 4 tq slices
TC = T // P     # 16 tk chunks
N_CORES = 8
FR = mybir.dt.float32r
F32 = mybir.dt.float32
AF = mybir.ActivationFunctionType


def build():
    nc = bacc.Bacc("TRN2", target_bir_lowering=False, debug=False, num_devices=N_CORES)
    xT = nc.dram_tensor("xT", [D, T], FR, kind="ExternalInput").ap()
    wqT = nc.dram_tensor("wqT", [D, F], FR, kind="ExternalInput").ap()
    wkT = nc.dram_tensor("wkT", [D, F], FR, kind="ExternalInput").ap()
    wvT = nc.dram_tensor("wvT", [D, F], FR, kind="ExternalInput").ap()
    woT = nc.dram_tensor("woT", [F, D], FR, kind="ExternalInput").ap()
    mask = nc.dram_tensor("mask", [P, 4 * 512], FR, kind="ExternalInput").ap()
    out = nc.dram_tensor("out", [T, D], F32, kind="ExternalOutput").ap()

    scale = 1.0 / np.sqrt(DH)

    with tile.TileContext(nc) as tc:
        with (
            tc.tile_pool(name="weights", bufs=1) as wpool,
            tc.tile_pool(name="persist", bufs=1) as persist,
            tc.tile_pool(name="x", bufs=2) as xpool,
            tc.tile_pool(name="sexp", bufs=6) as sepool,
            tc.tile_pool(name="small", bufs=4) as small,
            tc.tile_pool(name="outsb", bufs=4) as opool,
            tc.tile_pool(name="ps_s", bufs=3, space="PSUM") as ps_s,
            tc.tile_pool(name="ps_y", bufs=3, space="PSUM") as ps_y,
            tc.tile_pool(name="ps_ao", bufs=2, space="PSUM") as ps_ao,
        ):
            wq_sb = wpool.tile([P, DC, F], FR)
            wk_sb = wpool.tile([P, DC, F], FR)
            wv_sb = wpool.tile([P, DC, F], FR)
            wo_sb = wpool.tile([P, 2, D], FR)
            mask_sb = wpool.tile([P, 4, 512], FR)
            wq_r = wqT.rearrange("(o p) f -> p o f", p=P)
            wk_r = wkT.rearrange("(o p) f -> p o f", p=P)
            wv_r = wvT.rearrange("(o p) f -> p o f", p=P)
            wo_r = woT.rearrange("(g p) e -> p g e", p=P)
            xT_r = xT.rearrange("(o p) t -> p o t", p=P)

            qT_sb = persist.tile([P, 2, T], FR)
            kT_sb = persist.tile([P, 2, T], FR)
            v_sb = persist.tile([P, NH, TC, DH + 1], FR)
            yT_sb = persist.tile([P, 2, T], FR)

            x_tiles = {}

            def load_x(j):
                x_sb = xpool.tile([P, DC, 512], FR, tag="x", name=f"x_{j}")
                nc.sync.dma_start(x_sb[:], xT_r[:, :, 512 * j : 512 * (j + 1)])
                x_tiles[j] = x_sb

            # First projection's operands go first on the DMA queues, chunk
            # interleaved, so the first q matmul can start after one x+wq
            # chunk pair instead of after the whole 3MB preload.
            x0_sb = xpool.tile([P, DC, 512], FR, tag="x", name="x_0")
            x_tiles[0] = x0_sb
            for o in range(DC):
                nc.sync.dma_start(x0_sb[:, o], xT_r[:, o, 0:512])
                nc.sync.dma_start(wq_sb[:, o], wq_r[:, o])
            nc.sync.dma_start(wk_sb[:], wk_r[:])
            nc.sync.dma_start(wv_sb[:], wv_r[:])
            nc.sync.dma_start(mask_sb[:], mask.rearrange("p (r q) -> p r q", r=4))
            load_x(1)
            nc.sync.dma_start(wo_sb[:], wo_r[:])

            # V's ones column comes from the mask (memset can't write
            # float32r): mask[:, r=3, 511] is all ones.
            nc.vector.tensor_copy(
                v_sb[:, :, :, DH : DH + 1],
                mask_sb[:, 3:4, 511:512, None].to_broadcast([P, NH, TC, 1]),
            )

            def proj(j):
                jsl = slice(512 * j, 512 * (j + 1))
                x_sb = x_tiles[j]
                for w_sb, dst in ((wq_sb, qT_sb), (wk_sb, kT_sb)):
                    for c in range(2):
                        pt = ps_ao.tile([P, 512], F32, tag="ao", name=f"qk_{j}_{c}")
                        for o in range(DC):
                            nc.tensor.matmul(
                                pt[:],
                                w_sb[:, o, 128 * c : 128 * (c + 1)],
                                x_sb[:, o, :],
                                start=(o == 0),
                                stop=(o == DC - 1),
                            )
                        nc.scalar.copy(dst[:, c, jsl], pt[:])
                for i in range(4):
                    pt = ps_ao.tile([P, 512], F32, tag="ao", name=f"v_{j}_{i}")
                    for o in range(DC):
                        nc.tensor.matmul(
                            pt[:, :F],
                            x_sb[:, o, 128 * i : 128 * (i + 1)],
                            wv_sb[:, o, :],
                            start=(o == 0),
                            stop=(o == DC - 1),
                        )
                    nc.vector.tensor_copy(
                        v_sb[:, :, 4 * j + i, 0:DH],
                        pt[:, :F].rearrange("p (h d) -> p h d", h=NH),
                    )

            def outproj(j):
                for tb in range(4 * j, 4 * (j + 1)):
                    for eb in range(2):
                        pt = ps_ao.tile([P, 512], F32, tag="ao", name=f"o_{tb}_{eb}")
                        for g in range(2):
                            nc.tensor.matmul(
                                pt[:],
                                yT_sb[:, g, 128 * tb : 128 * (tb + 1)],
                                wo_sb[:, g, 512 * eb : 512 * (eb + 1)],
                                start=(g == 0),
                                stop=(g == 1),
                            )
                        osb = opool.tile([P, 512], F32, tag="osb", name=f"osb_{tb}_{eb}")
                        nc.vector.tensor_copy(osb[:], pt[:])
                        nc.sync.dma_start(
                            out[128 * tb : 128 * (tb + 1), 512 * eb : 512 * (eb + 1)],
                            osb[:],
                        )

            proj(0)
            for j in range(TJ):
                jsl = slice(512 * j, 512 * (j + 1))

                # ---- attention for tq-slice j, head pairs interleaved ----
                nii = 4 * j + 4
                for c in range(2):  # head pair (2c, 2c+1)
                    ypair = [
                        ps_y.tile([DH + 1, 512], F32, tag="y", name=f"y_{j}_{c}_{t}")
                        for t in range(2)
                    ]
                    for ii in range(nii):
                        r = ii - 4 * j
                        # Partial-column S^T matmul + exp skip the fully-masked
                        # left columns. j==0 stays full width so the first use
                        # of every se/sps pool slot writes finite data (stale
                        # slot contents are later multiplied by the 0-mask, so
                        # they must never be NaN). PV accumulation must be
                        # full width: a start=False matmul over a column
                        # subrange of a PSUM bank kills the exec unit.
                        col0 = 128 * r if (r > 0 and j > 0) else 0
                        sepair = []
                        for t in range(2):
                            hp = 64 * t
                            sps = ps_s.tile([P, 512], F32, tag="s")
                            nc.tensor.matmul(
                                sps[:, col0:],
                                kT_sb[hp : hp + DH, c, 128 * ii : 128 * (ii + 1)],
                                qT_sb[hp : hp + DH, c, 512 * j + col0 : 512 * (j + 1)],
                                start=True,
                                stop=True,
                            )
                            se = sepool.tile([P, 512], FR, tag="se")
                            nc.scalar.activation(
                                se[:, col0:], sps[:, col0:], AF.Exp, scale=scale
                            )
                            if col0 > 0:
                                # Fully-masked left columns: write explicit
                                # zeros (mask x 0) so every column the PV
                                # matmul reads was produced this iteration.
                                nc.vector.tensor_scalar_mul(
                                    se[:, 0:col0], mask_sb[:, 0, 0:col0], 0.0
                                )
                                nc.vector.tensor_mul(
                                    se[:, col0 : col0 + 128],
                                    se[:, col0 : col0 + 128],
                                    mask_sb[:, r, col0 : col0 + 128],
                                )
                            elif r >= 0:
                                # Full-width exp (j==0 or r==0): mask away
                                # everything left of the diagonal block.
                                nc.vector.tensor_mul(
                                    se[:, 0 : 128 * (r + 1)],
                                    se[:, 0 : 128 * (r + 1)],
                                    mask_sb[:, r, 0 : 128 * (r + 1)],
                                )
                            sepair.append(se)
                        for t in range(2):
                            h = 2 * c + t
                            nc.tensor.matmul(
                                ypair[t][:],
                                v_sb[:, h, ii, :],
                                sepair[t][:],
                                start=(ii == 0),
                                stop=(ii == nii - 1),
                            )
                    for t in range(2):
                        hp = 64 * t
                        yps = ypair[t]
                        rec = small.tile([1, 512], F32, tag="rec")
                        nc.vector.reciprocal(rec[:], yps[DH : DH + 1, :])
                        bsb = small.tile([DH, 512], F32, tag="bsb")
                        nc.gpsimd.partition_broadcast(bsb[:], rec[:])
                        nc.vector.tensor_mul(yT_sb[hp : hp + DH, c, jsl], yps[0:DH, :], bsb[:])

                # Next slice's projection goes ahead of the output projections:
                # it has no dependency on the norm chain above, so the PE has
                # work while the DVE finishes normalizing. Output projection of
                # slice j-1 (deps long satisfied) fills the same window; slice
                # j's own out-proj lands one iteration later.
                if j + 1 < TJ:
                    if j + 2 < TJ:
                        load_x(j + 2)
                    proj(j + 1)
                if j > 0:
                    outproj(j - 1)
                if j == TJ - 1:
                    outproj(j)
    nc.compile()
    return nc


def make_mask() -> np.ndarray:
    q = np.arange(512)[None, None, :]
    p = np.arange(P)[:, None, None]
    r = np.arange(4)[None, :, None]
    m = (q >= 128 * r + p).astype(np.float32)
    return np.ascontiguousarray(m.reshape(P, 4 * 512))


def shard_inputs(x, Wqkv, Wout):
    mask = make_mask()
    in_maps = []
    for c in range(N_CORES):
        b, g = c // 4, c % 4
        sl = slice(F * g, F * (g + 1))
        in_maps.append(
            {
                "xT": np.ascontiguousarray(x[b].T),
                "wqT": np.ascontiguousarray(Wqkv[sl, :].T),
                "wkT": np.ascontiguousarray(Wqkv[D:][sl, :].T),
                "wvT": np.ascontiguousarray(Wqkv[2 * D:][sl, :].T),
                "woT": np.ascontiguousarray(Wout[:, sl].T),
                "mask": mask,
            }
        )
    return in_maps


_NC_CACHE = None


def kernel(x, Wqkv, Wout):
    global _NC_CACHE
    x = np.asarray(x, dtype=np.float32)
    Wqkv = np.asarray(Wqkv, dtype=np.float32)
    Wout = np.asarray(Wout, dtype=np.float32)
    if _NC_CACHE is None:
        _NC_CACHE = build()
    nc = _NC_CACHE
    in_maps = shard_inputs(x, Wqkv, Wout)
    res = run_bass_kernel_spmd(nc, in_maps, core_ids=list(range(N_CORES)))
    outs = [res.results[c]["out"] for c in range(N_CORES)]
    return np.stack(
        [outs[0] + outs[1] + outs[2] + outs[3], outs[4] + outs[5] + outs[6] + outs[7]]
    )
